# revision 11
# baseline (speedup 1.0000x reference)
"""Cross-attention kernel for Trainium2, sharded head-parallel across 8 NeuronCores.

Problem: B=4, S=Sc=2048, D=1024, H=16, HD=64.
  q = rms_norm(x @ Wq + bq, gq)   per-head
  k = rms_norm(kv_k, gk), v = kv_v    from ctx @ Wkv + bkv
  out = softmax(q k^T / 8) v;   y = out @ Wproj + bproj

Sharding: tensor-parallel over heads. Each core owns 2 of the 16 heads: it
projects q/k/v for its heads only (column-sliced weights), runs attention,
and computes a partial y through its 128-row slice of Wproj. The host sums
the 8 partials and adds the bias terms (bproj and the v-bias, which passes
through attention as an additive constant since softmax rows sum to 1).

Device-side layout notes:
 - All matmul operands are float32r (TF32-like PE mode: full-rate at moving
   dim >= 256, ~1.6e-4 max rel err measured). Inputs stream from HBM via
   dtype-bitcast DMA (bit-identical to f32).
 - Scores are computed transposed, S^T[t, s] tiles, so that the exp'd
   probabilities sit t-on-partitions and feed the P^T-contraction of
   out^T = (v|1)^T @ P^T directly; the appended ones column yields the
   softmax denominators for free in row 64 of the output psum.
 - RMS-norm sums-of-squares are taken with a single matmul against a
   [128, 2] head-indicator matrix (carrying 1/g^2 so g can be folded into
   the weights on the host); 1/sqrt and the 1/8 score scale are folded
   into the k-side normalization multiply.

This walrus build only accepts ONE sync-wait per instruction; Tile emits
many. split_multi_waits() rewrites the serialized BIR, hoisting extra waits
onto single-wait NoOps on the same engine (in-order sequencers make that
equivalent).
"""

import sys

for _p in ("/opt/trn_rl_repo",):
    if _p not in sys.path:
        sys.path.insert(0, _p)

import numpy as np
import orjson

import concourse.bass as bass
import concourse.tile as tile
from concourse import mybir
from concourse.bass import ts, ds
from concourse.bass_utils import run_bass_kernel_spmd
from concourse.masks import make_identity

F32 = mybir.dt.float32
F32R = mybir.dt.float32r

B, S, SC, D, H, HD = 4, 2048, 2048, 1024, 16, 64
EPS = 1e-6
NCORES = 8
HPC = H // NCORES          # heads per core = 2
CW = HPC * HD              # per-core head-col width = 128
KT = D // 128              # contraction k-tiles = 8
NHALF = 2                  # token halves per batch
NCH = 4                    # 512-token chunks per batch
NSB = 4                    # 512-token s-blocks per batch
NTT = SC // 128            # 128-token t-tiles per batch = 16


def split_multi_waits(bir_json_bytes: bytes) -> bytes:
    m = orjson.loads(bir_json_bytes)
    n = [0]

    def fix_block(blk):
        insts = blk.get("instructions")
        if insts is None:
            return
        out = []
        for inst in insts:
            si = inst.get("sync_info")
            waits = (si or {}).get("on_wait") or []
            if len(waits) > 1:
                for w in waits[:-1]:
                    n[0] += 1
                    out.append({
                        "engine": inst.get("engine"),
                        "ins": [], "outs": [],
                        "name": f"WS-{n[0]}",
                        "opcode": "NoOp",
                        "sync_info": {"on_wait": [w], "on_update": []},
                    })
                si["on_wait"] = waits[-1:]
            out.append(inst)
        blk["instructions"] = out

    def walk(obj):
        if isinstance(obj, dict):
            if "instructions" in obj:
                fix_block(obj)
            for v in obj.values():
                walk(v)
        elif isinstance(obj, list):
            for v in obj:
                walk(v)

    for f in m["functions"]:
        walk(f.get("blocks", []))
    return orjson.dumps(m)


def build_bass():
    nc = bass.Bass("TRN2")

    XT = nc.dram_tensor("XT", [B, D, S], F32, kind="ExternalInput")
    CT = nc.dram_tensor("CT", [B, D, SC], F32, kind="ExternalInput")
    WQ = nc.dram_tensor("WQ", [D, CW], F32, kind="ExternalInput")
    WK = nc.dram_tensor("WK", [D, CW], F32, kind="ExternalInput")
    WV = nc.dram_tensor("WV", [D, CW], F32, kind="ExternalInput")
    WP = nc.dram_tensor("WP", [CW, D], F32, kind="ExternalInput")
    BQ = nc.dram_tensor("BQ", [CW, 1], F32, kind="ExternalInput")
    BK = nc.dram_tensor("BK", [CW, 1], F32, kind="ExternalInput")
    GQ = nc.dram_tensor("GQ", [CW, 2], F32, kind="ExternalInput")
    GK = nc.dram_tensor("GK", [CW, 2], F32, kind="ExternalInput")
    Y = nc.dram_tensor("Y", [B, S, D], F32, kind="ExternalOutput")

    with tile.TileContext(nc) as tc:
        with (
            tc.tile_pool(name="const", bufs=1) as const,
            tc.tile_pool(name="xin", bufs=1) as xin,
            tc.tile_pool(name="big", bufs=1) as big,
            tc.tile_pool(name="ework", bufs=2) as ework,
            tc.tile_pool(name="work", bufs=2) as work,
            tc.tile_pool(name="pproj", bufs=1, space="PSUM") as pproj,
            tc.tile_pool(name="pscore", bufs=2, space="PSUM") as pscore,
            tc.tile_pool(name="pout", bufs=1, space="PSUM") as pout,
        ):
            # ---- constants ----
            wq_sb = const.tile([128, KT, CW], F32R)
            wk_sb = const.tile([128, KT, CW], F32R)
            wv_sb = const.tile([128, KT, CW], F32R)
            nc.sync.dma_start(out=wq_sb, in_=WQ[:, :].rearrange("(k p) m -> p k m", p=128).bitcast(F32R))
            nc.sync.dma_start(out=wk_sb, in_=WK[:, :].rearrange("(k p) m -> p k m", p=128).bitcast(F32R))
            nc.sync.dma_start(out=wv_sb, in_=WV[:, :].rearrange("(k p) m -> p k m", p=128).bitcast(F32R))
            wp_sb = const.tile([CW, D], F32R)
            nc.sync.dma_start(out=wp_sb, in_=WP[:, :].bitcast(F32R))
            gq_sb = const.tile([CW, 2], F32R)
            gk_sb = const.tile([CW, 2], F32R)
            nc.sync.dma_start(out=gq_sb, in_=GQ[:, :].bitcast(F32R))
            nc.sync.dma_start(out=gk_sb, in_=GK[:, :].bitcast(F32R))
            bq_sb = const.tile([CW, 1], F32)
            bk_sb = const.tile([CW, 1], F32)
            nc.sync.dma_start(out=bq_sb, in_=BQ[:, :])
            nc.sync.dma_start(out=bk_sb, in_=BK[:, :])
            ident_f = const.tile([128, 128], F32)
            make_identity(nc, ident_f)
            ident = const.tile([128, 128], F32R)
            nc.vector.tensor_copy(out=ident, in_=ident_f)
            ones_row = const.tile([1, 128], F32)
            nc.vector.memset(ones_row, 1.0)
            epsq = const.tile([1, 1], F32)
            epsk = const.tile([1, 1], F32)
            nc.vector.memset(epsq, EPS)
            nc.vector.memset(epsk, 64.0 * EPS)

            for b in range(B):
                qT = big.tile([128, S], F32R, tag="qT")
                kT = big.tile([128, SC], F32R, tag="kT")
                vA = big.tile([128, NTT, 2 * (HD + 1)], F32R, tag="vA")
                oT = big.tile([128, S], F32R, tag="oT")
                # ones columns for the softmax denominators
                nc.vector.memset(vA[:, :, HD:HD + 1].bitcast(F32), 1.0)
                nc.vector.memset(vA[:, :, 2 * HD + 1:].bitcast(F32), 1.0)

                # ================= projections (per 1024-token half) ========
                for hf in range(NHALF):
                    xh = xin.tile([128, KT, 1024], F32R, tag="xh")
                    chh = xin.tile([128, KT, 1024], F32R, tag="ch")
                    for k in range(KT):
                        nc.sync.dma_start(out=xh[:, k, :], in_=XT[b, ts(k, 128), ts(hf, 1024)].bitcast(F32R))
                        nc.sync.dma_start(out=chh[:, k, :], in_=CT[b, ts(k, 128), ts(hf, 1024)].bitcast(F32R))

                    for cc in range(2):  # 512-token chunks within the half
                        ch = 2 * hf + cc
                        csl = ds(ch * 512, 512)          # chunk slice in b-token space
                        hsl = ds(cc * 512, 512)          # chunk slice within the half

                        # ---- q projection ----
                        psq = pproj.tile([128, 512], F32, tag="proj")
                        for k in range(KT):
                            nc.tensor.matmul(psq, wq_sb[:, k, :], xh[:, k, hsl],
                                             start=(k == 0), stop=(k == KT - 1))
                        nc.vector.tensor_scalar(out=qT[:, csl], in0=psq,
                                                scalar1=bq_sb[:, 0:1], scalar2=None,
                                                op0=mybir.AluOpType.add)
                        sq = work.tile([128, 512], F32R, tag="sq")
                        nc.vector.tensor_mul(out=sq, in0=qT[:, csl], in1=qT[:, csl])
                        rb = pproj.tile([128, 512], F32, tag="bcast")
                        for h in range(2):
                            pss = pproj.tile([1, 512], F32, tag="proj")
                            nc.tensor.matmul(pss, gq_sb[:, h:h + 1], sq, start=True, stop=True)
                            rs = work.tile([1, 512], F32, tag="rs")
                            nc.scalar.activation(out=rs, in_=pss,
                                                 func=mybir.ActivationFunctionType.Sqrt,
                                                 bias=epsq[:, 0:1], scale=1.0 / HD)
                            rr = work.tile([1, 512], F32, tag="rr")
                            nc.vector.reciprocal(out=rr, in_=rs)
                            nc.tensor.matmul(rb[ds(64 * h, 64), :], ones_row[0:1, 0:64], rr,
                                             start=True, stop=True, tile_position=(0, 64 * h))
                        nc.vector.tensor_mul(out=qT[:, csl], in0=qT[:, csl], in1=rb)

                        # ---- k projection ----
                        psk = pproj.tile([128, 512], F32, tag="proj")
                        for k in range(KT):
                            nc.tensor.matmul(psk, wk_sb[:, k, :], chh[:, k, hsl],
                                             start=(k == 0), stop=(k == KT - 1))
                        nc.vector.tensor_scalar(out=kT[:, csl], in0=psk,
                                                scalar1=bk_sb[:, 0:1], scalar2=None,
                                                op0=mybir.AluOpType.add)
                        sqk = work.tile([128, 512], F32R, tag="sq")
                        nc.vector.tensor_mul(out=sqk, in0=kT[:, csl], in1=kT[:, csl])
                        # 1/sqrt(sumsq + 64 eps) = 0.125 / sqrt(mean + eps): folds the
                        # softmax scale into the k normalization
                        rbk = pproj.tile([128, 512], F32, tag="bcast")
                        for h in range(2):
                            psks = pproj.tile([1, 512], F32, tag="proj")
                            nc.tensor.matmul(psks, gk_sb[:, h:h + 1], sqk, start=True, stop=True)
                            rsk = work.tile([1, 512], F32, tag="rs")
                            nc.scalar.activation(out=rsk, in_=psks,
                                                 func=mybir.ActivationFunctionType.Sqrt,
                                                 bias=epsk[:, 0:1], scale=1.0)
                            rrk = work.tile([1, 512], F32, tag="rr")
                            nc.vector.reciprocal(out=rrk, in_=rsk)
                            nc.tensor.matmul(rbk[ds(64 * h, 64), :], ones_row[0:1, 0:64], rrk,
                                             start=True, stop=True, tile_position=(0, 64 * h))
                        nc.vector.tensor_mul(out=kT[:, csl], in0=kT[:, csl], in1=rbk)

                        # ---- v projection (vT, then PE-transpose to [t, c]) ----
                        psv = pproj.tile([128, 512], F32, tag="proj")
                        for k in range(KT):
                            nc.tensor.matmul(psv, wv_sb[:, k, :], chh[:, k, hsl],
                                             start=(k == 0), stop=(k == KT - 1))
                        vts = work.tile([128, 512], F32R, tag="vts")
                        nc.vector.tensor_copy(out=vts, in_=psv)
                        for j in range(4):
                            tt = 4 * ch + j
                            pst = pproj.tile([128, 128], F32R, tag="proj")
                            nc.tensor.transpose(pst, vts[:, ts(j, 128)], ident)
                            nc.vector.tensor_copy(out=vA[:, tt, 0:HD], in_=pst[:, 0:HD])
                            nc.vector.tensor_copy(out=vA[:, tt, HD + 1:2 * HD + 1], in_=pst[:, HD:2 * HD])

                # ================= attention ================================
                for sb in range(NSB):
                    ssl = ts(sb, 512)
                    po0 = pout.tile([HD + 1, 512], F32, tag="o0")
                    po1 = pout.tile([HD + 1, 512], F32, tag="o1")
                    for tp in range(NTT // 2):
                        ps0 = pscore.tile([128, 2, 512], F32, tag="sc")
                        ps1 = pscore.tile([128, 2, 512], F32, tag="sc")
                        for j in range(2):
                            tt = 2 * tp + j
                            nc.tensor.matmul(ps0[:, j, :], kT[0:64, ts(tt, 128)], qT[0:64, ssl],
                                             start=True, stop=True, tile_position=(0, 0))
                            nc.tensor.matmul(ps1[:, j, :], kT[64:128, ts(tt, 128)], qT[64:128, ssl],
                                             start=True, stop=True, tile_position=(64, 0))
                        e0 = ework.tile([128, 2, 512], F32R, tag="e0")
                        e1 = ework.tile([128, 2, 512], F32R, tag="e1")
                        nc.scalar.activation(out=e0, in_=ps0, func=mybir.ActivationFunctionType.Exp)
                        nc.scalar.activation(out=e1, in_=ps1, func=mybir.ActivationFunctionType.Exp)
                        for j in range(2):
                            tt = 2 * tp + j
                            first = (tp == 0 and j == 0)
                            last = (tp == NTT // 2 - 1 and j == 1)
                            nc.tensor.matmul(po0, vA[:, tt, 0:HD + 1], e0[:, j, :],
                                             start=first, stop=last)
                            nc.tensor.matmul(po1, vA[:, tt, HD + 1:], e1[:, j, :],
                                             start=first, stop=last)
                    # normalize: rows 0:64 are sum(e*v), row 64 is sum(e)
                    rc0 = work.tile([1, 512], F32, tag="rc0")
                    rc1 = work.tile([1, 512], F32, tag="rc1")
                    nc.vector.reciprocal(out=rc0, in_=po0[HD:HD + 1, :])
                    nc.vector.reciprocal(out=rc1, in_=po1[HD:HD + 1, :])
                    nb = pproj.tile([128, 512], F32, tag="bcast")
                    nc.tensor.matmul(nb[0:64, :], ones_row[0:1, 0:64], rc0,
                                     start=True, stop=True, tile_position=(0, 0))
                    nc.tensor.matmul(nb[64:128, :], ones_row[0:1, 0:64], rc1,
                                     start=True, stop=True, tile_position=(0, 64))
                    nbs = work.tile([128, 512], F32, tag="nbs")
                    nc.vector.tensor_copy(out=nbs, in_=nb)
                    nc.vector.tensor_mul(out=oT[0:64, ssl], in0=po0[0:HD, :], in1=nbs[0:64, :])
                    # head 1 lands on partitions 0:64 of its psum; shift to 64:128 via DMA
                    o1s = work.tile([64, 512], F32R, tag="o1s")
                    nc.vector.tensor_mul(out=o1s, in0=po1[0:HD, :], in1=nbs[64:128, :])
                    nc.sync.dma_start(out=oT[64:128, ssl], in_=o1s)

                # ================= output projection ========================
                for st in range(S // 128):
                    ysb = work.tile([128, 1024], F32, tag="y")
                    for nb2 in range(2):
                        psy = pproj.tile([128, 512], F32, tag="proj")
                        nc.tensor.matmul(psy, oT[:, ts(st, 128)], wp_sb[:, ts(nb2, 512)],
                                         start=True, stop=True)
                        nc.vector.tensor_copy(out=ysb[:, ts(nb2, 512)], in_=psy)
                    nc.sync.dma_start(out=Y[b, ts(st, 128), :], in_=ysb)

    # one-wait-per-instruction workaround (see module docstring)
    orig_m = nc.m
    nc.to_json_bytes = lambda: split_multi_waits(mybir.module_to_json_bytes(orig_m))
    return nc


def host_inputs(x, ctx, Wq, bq, Wkv, bkv, Wproj, bproj, gq, gk):
    """Build the 8 per-core input maps (and remember host-side bias fixup)."""
    x = np.asarray(x, np.float32)
    ctx = np.asarray(ctx, np.float32)
    Wq = np.asarray(Wq, np.float32)
    bq = np.asarray(bq, np.float32)
    Wkv = np.asarray(Wkv, np.float32)
    bkv = np.asarray(bkv, np.float32)
    Wproj = np.asarray(Wproj, np.float32)
    gq = np.asarray(gq, np.float32)
    gk = np.asarray(gk, np.float32)

    XT = np.ascontiguousarray(x.transpose(0, 2, 1))
    CT = np.ascontiguousarray(ctx.transpose(0, 2, 1))

    gq_t = np.tile(gq, HPC)            # [CW]
    gk_t = np.tile(gk, HPC)
    ind = np.zeros((CW, 2), np.float32)

    in_maps = []
    for c in range(NCORES):
        heads = [HPC * c + i for i in range(HPC)]
        qcols = slice(c * CW, (c + 1) * CW)
        kcols = np.concatenate([np.arange(h * HD, (h + 1) * HD) for h in heads])
        vcols = kcols + D
        Wq_c = np.ascontiguousarray(Wq[:, qcols] * gq_t[None, :])
        bq_c = (bq[qcols] * gq_t).reshape(CW, 1)
        Wk_c = np.ascontiguousarray(Wkv[:, kcols] * gk_t[None, :])
        bk_c = (bkv[kcols] * gk_t).reshape(CW, 1)
        Wv_c = np.ascontiguousarray(Wkv[:, vcols])
        Wp_c = np.ascontiguousarray(Wproj[qcols, :])
        Gq = ind.copy()
        Gq[0:HD, 0] = 1.0 / np.maximum(gq, 1e-30) ** 2
        Gq[HD:CW, 1] = 1.0 / np.maximum(gq, 1e-30) ** 2
        Gk = ind.copy()
        Gk[0:HD, 0] = 1.0 / np.maximum(gk, 1e-30) ** 2
        Gk[HD:CW, 1] = 1.0 / np.maximum(gk, 1e-30) ** 2
        in_maps.append({
            "XT": XT, "CT": CT,
            "WQ": Wq_c, "WK": Wk_c, "WV": Wv_c, "WP": Wp_c,
            "BQ": np.ascontiguousarray(bq_c), "BK": np.ascontiguousarray(bk_c),
            "GQ": Gq, "GK": Gk,
        })

    # bias terms applied on the host after the partial sum:
    # y += bproj + bkv_v @ Wproj  (v-bias passes through softmax-weighted sum)
    bias = np.asarray(bproj, np.float32) + np.asarray(bkv, np.float32)[D:] @ Wproj
    return in_maps, bias


_CACHED = {}


def kernel(**inputs):
    if "nc" not in _CACHED:
        _CACHED["nc"] = build_bass()
    nc = _CACHED["nc"]
    in_maps, bias = host_inputs(**inputs)
    res = run_bass_kernel_spmd(nc, in_maps, core_ids=list(range(NCORES)), trace=False)
    out = np.zeros((B, S, D), np.float64)
    for r in res.results:
        out += r["Y"].astype(np.float64)
    out += bias.astype(np.float64)[None, None, :]
    return out.astype(np.float32)


# revision 12
# speedup vs baseline: 1.1178x; 1.1178x over previous
"""Cross-attention kernel for Trainium2, sharded head-parallel across 8 NeuronCores.

Problem: B=4, S=Sc=2048, D=1024, H=16, HD=64.
  q = rms_norm(x @ Wq + bq, gq)   per-head
  k = rms_norm(kv_k, gk), v = kv_v    from ctx @ Wkv + bkv
  out = softmax(q k^T / 8) v;   y = out @ Wproj + bproj

Sharding: tensor-parallel over heads. Each core owns 2 of the 16 heads: it
projects q/k/v for its heads only (column-sliced weights), runs attention,
and computes a partial y through its 128-row slice of Wproj. The host sums
the 8 partials and adds the bias terms (bproj and the v-bias, which passes
through attention as an additive constant since softmax rows sum to 1).

Device-side layout notes:
 - All matmul operands are float32r (TF32-like PE mode: full-rate at moving
   dim >= 256, ~1.6e-4 max rel err measured). Inputs stream from HBM via
   dtype-bitcast DMA (bit-identical to f32).
 - Scores are computed transposed, S^T[t, s] tiles, so that the exp'd
   probabilities sit t-on-partitions and feed the P^T-contraction of
   out^T = (v|1)^T @ P^T directly; the appended ones column yields the
   softmax denominators for free in row 64 of the output psum.
 - RMS-norm sums-of-squares are taken with a single matmul against a
   [128, 2] head-indicator matrix (carrying 1/g^2 so g can be folded into
   the weights on the host); 1/sqrt and the 1/8 score scale are folded
   into the k-side normalization multiply.

This walrus build only accepts ONE sync-wait per instruction; Tile emits
many. split_multi_waits() rewrites the serialized BIR, hoisting extra waits
onto single-wait NoOps on the same engine (in-order sequencers make that
equivalent).
"""

import sys

for _p in ("/opt/trn_rl_repo",):
    if _p not in sys.path:
        sys.path.insert(0, _p)

import numpy as np
import orjson

import concourse.bass as bass
import concourse.tile as tile
from concourse import mybir
from concourse.bass import ts, ds
from concourse.bass_utils import run_bass_kernel_spmd
from concourse.masks import make_identity

F32 = mybir.dt.float32
F32R = mybir.dt.float32r
BF16 = mybir.dt.bfloat16

B, S, SC, D, H, HD = 4, 2048, 2048, 1024, 16, 64
EPS = 1e-6
NCORES = 8
HPC = H // NCORES          # heads per core = 2
CW = HPC * HD              # per-core head-col width = 128
KT = D // 128              # contraction k-tiles = 8
NHALF = 2                  # token halves per batch
NCH = 4                    # 512-token chunks per batch
NSB = 4                    # 512-token s-blocks per batch
NTT = SC // 128            # 128-token t-tiles per batch = 16


def split_multi_waits(bir_json_bytes: bytes) -> bytes:
    m = orjson.loads(bir_json_bytes)
    n = [0]

    def fix_block(blk):
        insts = blk.get("instructions")
        if insts is None:
            return
        out = []
        for inst in insts:
            si = inst.get("sync_info")
            waits = (si or {}).get("on_wait") or []
            if len(waits) > 1:
                for w in waits[:-1]:
                    n[0] += 1
                    out.append({
                        "engine": inst.get("engine"),
                        "ins": [], "outs": [],
                        "name": f"WS-{n[0]}",
                        "opcode": "NoOp",
                        "sync_info": {"on_wait": [w], "on_update": []},
                    })
                si["on_wait"] = waits[-1:]
            out.append(inst)
        blk["instructions"] = out

    def walk(obj):
        if isinstance(obj, dict):
            if "instructions" in obj:
                fix_block(obj)
            for v in obj.values():
                walk(v)
        elif isinstance(obj, list):
            for v in obj:
                walk(v)

    for f in m["functions"]:
        walk(f.get("blocks", []))
    return orjson.dumps(m)


def build_bass():
    nc = bass.Bass("TRN2")

    XT = nc.dram_tensor("XT", [B, D, S], F32, kind="ExternalInput")
    CT = nc.dram_tensor("CT", [B, D, SC], F32, kind="ExternalInput")
    WQ = nc.dram_tensor("WQ", [D, CW], F32, kind="ExternalInput")
    WK = nc.dram_tensor("WK", [D, CW], F32, kind="ExternalInput")
    WV = nc.dram_tensor("WV", [D, CW], F32, kind="ExternalInput")
    WP = nc.dram_tensor("WP", [CW, D], F32, kind="ExternalInput")
    BQ = nc.dram_tensor("BQ", [CW, 1], F32, kind="ExternalInput")
    BK = nc.dram_tensor("BK", [CW, 1], F32, kind="ExternalInput")
    GQ = nc.dram_tensor("GQ", [CW, 2], F32, kind="ExternalInput")
    GK = nc.dram_tensor("GK", [CW, 2], F32, kind="ExternalInput")
    Y = nc.dram_tensor("Y", [B, S, D], F32, kind="ExternalOutput")

    with tile.TileContext(nc) as tc:
        with (
            tc.tile_pool(name="const", bufs=1) as const,
            tc.tile_pool(name="xin", bufs=1) as xin,
            tc.tile_pool(name="big", bufs=1) as big,
            tc.tile_pool(name="ework", bufs=2) as ework,
            tc.tile_pool(name="work", bufs=2) as work,
            tc.tile_pool(name="pproj", bufs=1, space="PSUM") as pproj,
            tc.tile_pool(name="pscore", bufs=2, space="PSUM") as pscore,
            tc.tile_pool(name="pout", bufs=1, space="PSUM") as pout,
        ):
            # ---- constants ----
            wq_sb = const.tile([128, KT, CW], F32R)
            wk_sb = const.tile([128, KT, CW], F32R)
            wv_sb = const.tile([128, KT, CW], F32R)
            nc.sync.dma_start(out=wq_sb, in_=WQ[:, :].rearrange("(k p) m -> p k m", p=128).bitcast(F32R))
            nc.sync.dma_start(out=wk_sb, in_=WK[:, :].rearrange("(k p) m -> p k m", p=128).bitcast(F32R))
            nc.sync.dma_start(out=wv_sb, in_=WV[:, :].rearrange("(k p) m -> p k m", p=128).bitcast(F32R))
            wp_sb = const.tile([CW, D], BF16)
            nc.gpsimd.dma_start(out=wp_sb, in_=WP[:, :])
            gq_sb = const.tile([CW, 2], BF16)
            gk_sb = const.tile([CW, 2], BF16)
            nc.gpsimd.dma_start(out=gq_sb, in_=GQ[:, :])
            nc.gpsimd.dma_start(out=gk_sb, in_=GK[:, :])
            bq_sb = const.tile([CW, 1], F32)
            bk_sb = const.tile([CW, 1], F32)
            nc.sync.dma_start(out=bq_sb, in_=BQ[:, :])
            nc.sync.dma_start(out=bk_sb, in_=BK[:, :])
            ident_f = const.tile([128, 128], F32)
            make_identity(nc, ident_f)
            ident = const.tile([128, 128], F32R)
            nc.vector.tensor_copy(out=ident, in_=ident_f)
            ones_row = const.tile([1, 128], F32)
            nc.vector.memset(ones_row, 1.0)
            epsq = const.tile([1, 1], F32)
            epsk = const.tile([1, 1], F32)
            nc.vector.memset(epsq, EPS)
            nc.vector.memset(epsk, 64.0 * EPS)

            for b in range(B):
                qT = big.tile([128, S], BF16, tag="qT")
                kT = big.tile([128, SC], BF16, tag="kT")
                vA = big.tile([128, NTT, 2 * (HD + 1)], BF16, tag="vA")
                oT = big.tile([128, S], BF16, tag="oT")
                # ones columns for the softmax denominators
                nc.vector.memset(vA[:, :, HD:HD + 1], 1.0)
                nc.vector.memset(vA[:, :, 2 * HD + 1:], 1.0)

                # ================= projections (per 1024-token half) ========
                for hf in range(NHALF):
                    xh = xin.tile([128, KT, 1024], F32R, tag="xh")
                    chh = xin.tile([128, KT, 1024], F32R, tag="ch")
                    for k in range(KT):
                        nc.sync.dma_start(out=xh[:, k, :], in_=XT[b, ts(k, 128), ts(hf, 1024)].bitcast(F32R))
                        nc.sync.dma_start(out=chh[:, k, :], in_=CT[b, ts(k, 128), ts(hf, 1024)].bitcast(F32R))

                    for cc in range(2):  # 512-token chunks within the half
                        ch = 2 * hf + cc
                        csl = ds(ch * 512, 512)          # chunk slice in b-token space
                        hsl = ds(cc * 512, 512)          # chunk slice within the half

                        # ---- q projection ----
                        psq = pproj.tile([128, 512], F32, tag="proj")
                        for k in range(KT):
                            nc.tensor.matmul(psq, wq_sb[:, k, :], xh[:, k, hsl],
                                             start=(k == 0), stop=(k == KT - 1))
                        nc.vector.tensor_scalar(out=qT[:, csl], in0=psq,
                                                scalar1=bq_sb[:, 0:1], scalar2=None,
                                                op0=mybir.AluOpType.add)
                        sq = work.tile([128, 512], BF16, tag="sq")
                        nc.vector.tensor_mul(out=sq, in0=qT[:, csl], in1=qT[:, csl])
                        rb = pproj.tile([128, 512], F32, tag="bcast")
                        for h in range(2):
                            pss = pproj.tile([1, 512], F32, tag="proj")
                            nc.tensor.matmul(pss, gq_sb[:, h:h + 1], sq, start=True, stop=True)
                            rs = work.tile([1, 512], F32, tag="rs")
                            nc.scalar.activation(out=rs, in_=pss,
                                                 func=mybir.ActivationFunctionType.Sqrt,
                                                 bias=epsq[:, 0:1], scale=1.0 / HD)
                            rr = work.tile([1, 512], F32, tag="rr")
                            nc.vector.reciprocal(out=rr, in_=rs)
                            nc.tensor.matmul(rb[ds(64 * h, 64), :], ones_row[0:1, 0:64], rr,
                                             start=True, stop=True, tile_position=(0, 64 * h))
                        nc.vector.tensor_mul(out=qT[:, csl], in0=qT[:, csl], in1=rb)

                        # ---- k projection ----
                        psk = pproj.tile([128, 512], F32, tag="proj")
                        for k in range(KT):
                            nc.tensor.matmul(psk, wk_sb[:, k, :], chh[:, k, hsl],
                                             start=(k == 0), stop=(k == KT - 1))
                        nc.vector.tensor_scalar(out=kT[:, csl], in0=psk,
                                                scalar1=bk_sb[:, 0:1], scalar2=None,
                                                op0=mybir.AluOpType.add)
                        sqk = work.tile([128, 512], BF16, tag="sq")
                        nc.vector.tensor_mul(out=sqk, in0=kT[:, csl], in1=kT[:, csl])
                        # 1/sqrt(sumsq + 64 eps) = 0.125 / sqrt(mean + eps): folds the
                        # softmax scale into the k normalization
                        rbk = pproj.tile([128, 512], F32, tag="bcast")
                        for h in range(2):
                            psks = pproj.tile([1, 512], F32, tag="proj")
                            nc.tensor.matmul(psks, gk_sb[:, h:h + 1], sqk, start=True, stop=True)
                            rsk = work.tile([1, 512], F32, tag="rs")
                            nc.scalar.activation(out=rsk, in_=psks,
                                                 func=mybir.ActivationFunctionType.Sqrt,
                                                 bias=epsk[:, 0:1], scale=1.0)
                            rrk = work.tile([1, 512], F32, tag="rr")
                            nc.vector.reciprocal(out=rrk, in_=rsk)
                            nc.tensor.matmul(rbk[ds(64 * h, 64), :], ones_row[0:1, 0:64], rrk,
                                             start=True, stop=True, tile_position=(0, 64 * h))
                        nc.vector.tensor_mul(out=kT[:, csl], in0=kT[:, csl], in1=rbk)

                        # ---- v projection (vT, then PE-transpose to [t, c]) ----
                        psv = pproj.tile([128, 512], F32, tag="proj")
                        for k in range(KT):
                            nc.tensor.matmul(psv, wv_sb[:, k, :], chh[:, k, hsl],
                                             start=(k == 0), stop=(k == KT - 1))
                        vts = work.tile([128, 512], F32R, tag="vts")
                        nc.vector.tensor_copy(out=vts, in_=psv)
                        for j in range(4):
                            tt = 4 * ch + j
                            pst = pproj.tile([128, 128], F32R, tag="proj")
                            nc.tensor.transpose(pst, vts[:, ts(j, 128)], ident)
                            nc.vector.tensor_copy(out=vA[:, tt, 0:HD], in_=pst[:, 0:HD])
                            nc.vector.tensor_copy(out=vA[:, tt, HD + 1:2 * HD + 1], in_=pst[:, HD:2 * HD])

                # ================= attention ================================
                for sb in range(NSB):
                    ssl = ts(sb, 512)
                    po0 = pout.tile([HD + 1, 512], F32, tag="o0")
                    po1 = pout.tile([HD + 1, 512], F32, tag="o1")
                    for tp in range(NTT // 2):
                        ps0 = pscore.tile([128, 2, 512], F32, tag="sc")
                        ps1 = pscore.tile([128, 2, 512], F32, tag="sc")
                        for j in range(2):
                            tt = 2 * tp + j
                            nc.tensor.matmul(ps0[:, j, :], kT[0:64, ts(tt, 128)], qT[0:64, ssl],
                                             start=True, stop=True, tile_position=(0, 0))
                            nc.tensor.matmul(ps1[:, j, :], kT[64:128, ts(tt, 128)], qT[64:128, ssl],
                                             start=True, stop=True, tile_position=(64, 0))
                        e0 = ework.tile([128, 2, 512], BF16, tag="e0")
                        e1 = ework.tile([128, 2, 512], BF16, tag="e1")
                        nc.scalar.activation(out=e0, in_=ps0, func=mybir.ActivationFunctionType.Exp)
                        nc.scalar.activation(out=e1, in_=ps1, func=mybir.ActivationFunctionType.Exp)
                        for j in range(2):
                            tt = 2 * tp + j
                            first = (tp == 0 and j == 0)
                            last = (tp == NTT // 2 - 1 and j == 1)
                            nc.tensor.matmul(po0, vA[:, tt, 0:HD + 1], e0[:, j, :],
                                             start=first, stop=last)
                            nc.tensor.matmul(po1, vA[:, tt, HD + 1:], e1[:, j, :],
                                             start=first, stop=last)
                    # normalize: rows 0:64 are sum(e*v), row 64 is sum(e)
                    rc0 = work.tile([1, 512], F32, tag="rc0")
                    rc1 = work.tile([1, 512], F32, tag="rc1")
                    nc.vector.reciprocal(out=rc0, in_=po0[HD:HD + 1, :])
                    nc.vector.reciprocal(out=rc1, in_=po1[HD:HD + 1, :])
                    nb = pproj.tile([128, 512], F32, tag="bcast")
                    nc.tensor.matmul(nb[0:64, :], ones_row[0:1, 0:64], rc0,
                                     start=True, stop=True, tile_position=(0, 0))
                    nc.tensor.matmul(nb[64:128, :], ones_row[0:1, 0:64], rc1,
                                     start=True, stop=True, tile_position=(0, 64))
                    nbs = work.tile([128, 512], F32, tag="nbs")
                    nc.vector.tensor_copy(out=nbs, in_=nb)
                    nc.vector.tensor_mul(out=oT[0:64, ssl], in0=po0[0:HD, :], in1=nbs[0:64, :])
                    # head 1 lands on partitions 0:64 of its psum; shift to 64:128 via DMA
                    o1s = work.tile([64, 512], BF16, tag="o1s")
                    nc.vector.tensor_mul(out=o1s, in0=po1[0:HD, :], in1=nbs[64:128, :])
                    nc.sync.dma_start(out=oT[64:128, ssl], in_=o1s)

                # ================= output projection ========================
                for st in range(S // 128):
                    ysb = work.tile([128, 1024], F32, tag="y")
                    for nb2 in range(2):
                        psy = pproj.tile([128, 512], F32, tag="proj")
                        nc.tensor.matmul(psy, oT[:, ts(st, 128)], wp_sb[:, ts(nb2, 512)],
                                         start=True, stop=True)
                        nc.vector.tensor_copy(out=ysb[:, ts(nb2, 512)], in_=psy)
                    nc.sync.dma_start(out=Y[b, ts(st, 128), :], in_=ysb)

    # one-wait-per-instruction workaround (see module docstring)
    orig_m = nc.m
    nc.to_json_bytes = lambda: split_multi_waits(mybir.module_to_json_bytes(orig_m))
    return nc


def host_inputs(x, ctx, Wq, bq, Wkv, bkv, Wproj, bproj, gq, gk):
    """Build the 8 per-core input maps (and remember host-side bias fixup)."""
    x = np.asarray(x, np.float32)
    ctx = np.asarray(ctx, np.float32)
    Wq = np.asarray(Wq, np.float32)
    bq = np.asarray(bq, np.float32)
    Wkv = np.asarray(Wkv, np.float32)
    bkv = np.asarray(bkv, np.float32)
    Wproj = np.asarray(Wproj, np.float32)
    gq = np.asarray(gq, np.float32)
    gk = np.asarray(gk, np.float32)

    XT = np.ascontiguousarray(x.transpose(0, 2, 1))
    CT = np.ascontiguousarray(ctx.transpose(0, 2, 1))

    gq_t = np.tile(gq, HPC)            # [CW]
    gk_t = np.tile(gk, HPC)
    ind = np.zeros((CW, 2), np.float32)

    in_maps = []
    for c in range(NCORES):
        heads = [HPC * c + i for i in range(HPC)]
        qcols = slice(c * CW, (c + 1) * CW)
        kcols = np.concatenate([np.arange(h * HD, (h + 1) * HD) for h in heads])
        vcols = kcols + D
        Wq_c = np.ascontiguousarray(Wq[:, qcols] * gq_t[None, :])
        bq_c = (bq[qcols] * gq_t).reshape(CW, 1)
        Wk_c = np.ascontiguousarray(Wkv[:, kcols] * gk_t[None, :])
        bk_c = (bkv[kcols] * gk_t).reshape(CW, 1)
        Wv_c = np.ascontiguousarray(Wkv[:, vcols])
        Wp_c = np.ascontiguousarray(Wproj[qcols, :])
        Gq = ind.copy()
        Gq[0:HD, 0] = 1.0 / np.maximum(gq, 1e-30) ** 2
        Gq[HD:CW, 1] = 1.0 / np.maximum(gq, 1e-30) ** 2
        Gk = ind.copy()
        Gk[0:HD, 0] = 1.0 / np.maximum(gk, 1e-30) ** 2
        Gk[HD:CW, 1] = 1.0 / np.maximum(gk, 1e-30) ** 2
        in_maps.append({
            "XT": XT, "CT": CT,
            "WQ": Wq_c, "WK": Wk_c, "WV": Wv_c, "WP": Wp_c,
            "BQ": np.ascontiguousarray(bq_c), "BK": np.ascontiguousarray(bk_c),
            "GQ": Gq, "GK": Gk,
        })

    # bias terms applied on the host after the partial sum:
    # y += bproj + bkv_v @ Wproj  (v-bias passes through softmax-weighted sum)
    bias = np.asarray(bproj, np.float32) + np.asarray(bkv, np.float32)[D:] @ Wproj
    return in_maps, bias


_CACHED = {}


def kernel(**inputs):
    if "nc" not in _CACHED:
        _CACHED["nc"] = build_bass()
    nc = _CACHED["nc"]
    in_maps, bias = host_inputs(**inputs)
    res = run_bass_kernel_spmd(nc, in_maps, core_ids=list(range(NCORES)), trace=False)
    out = np.zeros((B, S, D), np.float64)
    for r in res.results:
        out += r["Y"].astype(np.float64)
    out += bias.astype(np.float64)[None, None, :]
    return out.astype(np.float32)
